# revision 9
# baseline (speedup 1.0000x reference)
"""VQ codebook lookup (nn_VQ) on 8 TRN2 NeuronCores.

reference: idx = argmin_k ||x_n - e_k||^2 ; out = embeddings[idx]
Equivalent: idx = argmax_k (x_n . e_k - 0.5||e_k||^2)  (||x||^2 is constant per row)

Strategy v2 (data-parallel over N, codebook replicated):
  - Host: per core, shard x into [62500, 100] f32, pad to [62976, 100]
    (123 super-tiles of 512 rows), pre-transpose each super-tile to
    [101, 512] f32 (row 100 = 1.0 bias-aug) with the 512 columns stored in
    interleaved order (device col c*128+p holds original row 4p+c) so the
    output DMA sees 1600B-contiguous elements. Codebook constants:
    eth [101, 256] = [e.T; -0.5||e||^2] zero-padded to 256 moving cols,
    e2 [100, 256] = e zero-padded, identity [128, 128].
  - Device, per 512-row super-tile (all matmuls are single-instruction
    self-loading f32/f32r; 12 PE instructions per tile):
      scores: 4x f32r matmul (xT chunk [101,128] stationary, eth moving,
        256-wide so f32r runs at 1 cycle/row) -> PSUM [128, 4, 256]
      DVE reduce_max + is_ge (broadcast AP) -> exact one-hot f32 mask
      4x PE-transpose (f32r, identity moving) -> maskT PSUM [100, 4, 128]
      Act copy -> maskTs SBUF
      gather: 4x f32r matmul (maskT chunk stationary, e2 [100, 256] moving,
        1 cycle/row) -> out rows PSUM [128, 4, 256]
      Pool (gpsimd) copy [:, :, 0:100] into a 3-tile output group buffer;
      one DMA out per 3 tiles (grouped, 1600B elems via the interleave).
  - DMAs: input loads grouped 3 super-tiles per instruction (plain loads of
    the host-pretransposed layout; elem = 2KB).
  - Software pipeline: per iteration i emit S(i), RM(i-1), TC(i-2), C2(i-4),
    G(i-3) so the PE never stalls on the DVE argmax or the copies.
"""

import sys

sys.path.insert(0, "/opt/trn_rl_repo")
from contextlib import ExitStack

import ml_dtypes
import numpy as np

import concourse.bass as bass
import concourse.bacc as bacc
import concourse.tile as tile
from concourse import mybir
from concourse._compat import with_exitstack
from concourse.bass_utils import run_bass_kernel_spmd

F32 = mybir.dt.float32
F32R = mybir.dt.float32r
BF = mybir.dt.bfloat16

N_TOTAL = 500_000
D = 100
K = 100
N_CORES = 8
ST = 512  # rows per super-tile
N_SHARD = N_TOTAL // N_CORES  # 62500
T = -(-N_SHARD // ST)  # 123 super-tiles
NP = T * ST  # 62976 padded rows per core
GIN = 3  # super-tiles per input DMA
GOUT = 3  # super-tiles per output DMA
NG = T // GIN  # 41 groups (123 = 41*3)
KP = 256  # moving-dim padding for 1-cycle f32r matmuls


@with_exitstack
def _vq_tile_kernel(ctx: ExitStack, tc: tile.TileContext, out, xt, eth, e2, ident):
    nc = tc.nc

    consts = ctx.enter_context(tc.tile_pool(name="consts", bufs=1))
    eth_s = consts.tile([101, K], F32, tag="eth")
    nc.sync.dma_start(eth_s[:], eth[:])
    e2b_s = consts.tile([K, D], BF, tag="e2b")
    nc.sync.dma_start(e2b_s[:], e2[:])
    id_s = consts.tile([128, 128], BF, tag="ident")
    nc.sync.dma_start(id_s[:], ident[:])

    xp = ctx.enter_context(tc.tile_pool(name="xt", bufs=3))
    sp = ctx.enter_context(tc.tile_pool(name="scores", bufs=3, space="PSUM"))
    mvp = ctx.enter_context(tc.tile_pool(name="maxv", bufs=2))
    mp = ctx.enter_context(tc.tile_pool(name="mask", bufs=2))
    mtp = ctx.enter_context(tc.tile_pool(name="maskT", bufs=1, space="PSUM"))
    msp = ctx.enter_context(tc.tile_pool(name="maskTs", bufs=2))
    opp = ctx.enter_context(tc.tile_pool(name="outp", bufs=1, space="PSUM"))
    ogp = ctx.enter_context(tc.tile_pool(name="outg", bufs=2))

    xt_v = xt.rearrange("(g u) d n -> g d u n", u=GIN)  # [41, 101, 3, 512]
    out_v = out.rearrange("(g u p w) d -> g p u w d", u=GOUT, p=128, w=4)

    xtiles = {}
    scores_t = {}
    mask_t = {}
    maskTs_t = {}
    outp_t = {}
    outg_t = {}

    def load(g):
        tl = xp.tile([101, GIN, ST], F32, tag="xt")
        nc.sync.dma_start(out=tl[:], in_=xt_v[g])
        xtiles[g] = tl

    def S(t):
        g, u = divmod(t, GIN)
        tl = xtiles[g]
        sc = sp.tile([128, 4, K], F32, tag="scores")
        for c in range(4):
            nc.tensor.matmul(
                sc[:, c], tl[:, u, bass.ts(c, 128)], eth_s[:], start=True, stop=True
            )
        scores_t[t] = sc
        if u == GIN - 1:
            del xtiles[g]

    def RM(t):
        sc = scores_t.pop(t)
        mv = mvp.tile([128, 4], F32, tag="maxv")
        nc.vector.tensor_reduce(
            mv[:], sc[:], axis=mybir.AxisListType.X, op=mybir.AluOpType.max
        )
        mk = mp.tile([128, 4, K], BF, tag="mask")
        mvv = mv[:].rearrange("p (f o) -> p f o", o=1)
        s_ap, m_ap = bass.broadcast_tensor_aps(sc[:], mvv)
        nc.vector.tensor_tensor(out=mk[:], in0=s_ap, in1=m_ap, op=mybir.AluOpType.is_ge)
        mask_t[t] = mk

    def TC(t):
        mk = mask_t.pop(t)
        mt = mtp.tile([K, 4, 128], BF, tag="maskT")
        for c in range(4):
            nc.tensor.transpose(mt[:, c], mk[:, c], id_s[:])
        ms = msp.tile([K, 4, 128], BF, tag="maskTs")
        nc.scalar.copy(ms[:], mt[:])
        maskTs_t[t] = ms

    def G(t):
        ms = maskTs_t.pop(t)
        op_ = opp.tile([128, 4, D], F32, tag="outp")
        for c in range(4):
            nc.tensor.matmul(
                op_[:, c], ms[:, c], e2b_s[:], start=True, stop=True
            )
        outp_t[t] = op_

    def C2(t):
        g, u = divmod(t, GOUT)
        if u == 0:
            outg_t[g] = ogp.tile([128, GOUT, 4, D], BF, tag="outg", name="outg")
        og = outg_t[g]
        op_ = outp_t.pop(t)
        nc.scalar.copy(og[:, u], op_[:, :, :])
        if u == GOUT - 1:
            nc.sync.dma_start(out=out_v[g], in_=og[:])
            del outg_t[g]

    load(0)
    load(1)
    for i in range(T + 4):
        if i % GIN == 0:
            g = i // GIN + 2
            if g < NG:
                load(g)
        if i < T:
            S(i)
        if 0 <= i - 1 < T:
            RM(i - 1)
        if 0 <= i - 2 < T:
            TC(i - 2)
        if 0 <= i - 4 < T:
            C2(i - 4)
        if 0 <= i - 3 < T:
            G(i - 3)


def build_nc():
    nc = bacc.Bacc(
        "TRN2",
        target_bir_lowering=False,
        debug=False,
        enable_asserts=True,
        num_devices=N_CORES,
    )
    out = nc.dram_tensor("out", [NP, D], BF, kind="ExternalOutput").ap()
    xt = nc.dram_tensor("xt", [T, 101, ST], F32, kind="ExternalInput").ap()
    eth = nc.dram_tensor("eth", [101, K], F32, kind="ExternalInput").ap()
    e2 = nc.dram_tensor("e2", [K, D], BF, kind="ExternalInput").ap()
    ident = nc.dram_tensor("ident", [128, 128], BF, kind="ExternalInput").ap()
    with tile.TileContext(nc) as tc:
        _vq_tile_kernel(tc, out, xt, eth, e2, ident)
    nc.compile()
    return nc


def prep_inputs(inputs: np.ndarray, embeddings: np.ndarray):
    """Host-side shard + layout prep. Returns in_maps for the 8 cores."""
    x = np.ascontiguousarray(inputs, dtype=np.float32)
    e = np.ascontiguousarray(embeddings, dtype=np.float32)

    eth = np.zeros((101, K), dtype=np.float32)
    eth[0:D] = e.T
    eth[D] = (-0.5 * np.sum(e.astype(np.float64) ** 2, axis=1)).astype(np.float32)
    e2 = e.astype(ml_dtypes.bfloat16)
    ident = np.eye(128, dtype=ml_dtypes.bfloat16)

    in_maps = []
    for i in range(N_CORES):
        xs = x[i * N_SHARD : (i + 1) * N_SHARD]
        xpad = np.zeros((NP, D), dtype=np.float32)
        xpad[:N_SHARD] = xs
        # device col c*128+p holds original row 4p+c of the super-tile
        v = xpad.reshape(T, 128, 4, D)
        xt = np.empty((T, 101, ST), dtype=np.float32)
        xt[:, 0:D, :] = v.transpose(0, 3, 2, 1).reshape(T, D, ST)
        xt[:, D, :] = 1.0
        in_maps.append({"xt": xt, "eth": eth, "e2": e2, "ident": ident})
    return in_maps


_NC_CACHE = None


def kernel(inputs: np.ndarray, embeddings: np.ndarray) -> np.ndarray:
    global _NC_CACHE
    if _NC_CACHE is None:
        _NC_CACHE = build_nc()
    nc = _NC_CACHE
    in_maps = prep_inputs(inputs, embeddings)
    res = run_bass_kernel_spmd(nc, in_maps, core_ids=list(range(N_CORES)))
    shards = [res.results[i]["out"][:N_SHARD] for i in range(N_CORES)]
    full = np.concatenate(shards, axis=0)
    return np.ascontiguousarray(full.astype(np.float32))


# revision 14
# speedup vs baseline: 1.2186x; 1.2186x over previous
"""VQ codebook lookup (nn_VQ) on 8 TRN2 NeuronCores.

reference: idx = argmin_k ||x_n - e_k||^2 ; out = embeddings[idx]
Equivalent: idx = argmax_k (x_n . e_k - 0.5||e_k||^2)  (||x||^2 is constant per row)

Strategy v4 (data-parallel over N, codebook replicated):
  - Host: per core, shard x into [62500, 100] f32, pad to [63488, 100]
    (62 super-tiles of 1024 rows), pre-transpose each super-tile to
    [101, 1024] (row 100 = 1.0 bias-aug) with the 1024 columns stored in
    interleaved order (device col c*128+p holds original row 8p+c) so the
    output DMA sees contiguous multi-row elements, then split into bf16
    hi/lo halves: xt [62, 2, 101, 1024] bf16. Codebook constants:
    eth_h/eth_l [101, 100] bf16 hi/lo of [e.T; -0.5||e||^2],
    e2 [100, 100] bf16, identity [128, 128] bf16.
  - Device, per 1024-row super-tile:
      scores: 8 chunks x 3 accumulating bf16 matmuls (x_hi.eth_h + x_hi.eth_l
        + x_lo.eth_h) -> PSUM [128, 8, 128] f32 (chunks strided 512B to stay
        bank-aligned, 100 real cols each)
      DVE reduce_max + is_ge (broadcast AP) -> exact one-hot bf16 mask
      8x PE-transpose (bf16, identity moving) -> maskT PSUM [100, 8, 128]
      Act copy -> maskTs SBUF
      gather: 8x bf16 matmul (maskT chunk stationary, e2 moving)
        -> out rows PSUM [128, 8, 128] f32 (512B-strided chunks)
      Act copy (casts f32->bf16) into a 2-tile output group buffer;
      one DMA out per 2 tiles (contiguous 1600B elems via the interleave).
  - DMAs: input loads grouped 2 super-tiles per instruction (plain loads of
    the host-pretransposed layout; elem = 2KB). Output bf16, upcast on host.
  - Software pipeline: per iteration i emit S(i), RM(i-1), TC(i-2), C2(i-4),
    G(i-3) so the PE never stalls on the DVE argmax or the copies.
"""

import sys

sys.path.insert(0, "/opt/trn_rl_repo")
from contextlib import ExitStack

import ml_dtypes
import numpy as np

import concourse.bass as bass
import concourse.bacc as bacc
import concourse.tile as tile
from concourse import mybir
from concourse._compat import with_exitstack
from concourse.bass_utils import run_bass_kernel_spmd

F32 = mybir.dt.float32
BF = mybir.dt.bfloat16
bf16 = ml_dtypes.bfloat16

N_TOTAL = 500_000
D = 100
K = 100
N_CORES = 8
ST = 1024  # rows per super-tile
NCH = ST // 128  # 8 chunks
N_SHARD = N_TOTAL // N_CORES  # 62500
T = -(-N_SHARD // ST)  # 62 super-tiles
NP = T * ST  # 63488 padded rows per core
GIN = 2  # super-tiles per input DMA
GOUT = 2  # super-tiles per output DMA
NG = T // GIN  # 31 groups


@with_exitstack
def _vq_tile_kernel(ctx: ExitStack, tc: tile.TileContext, out, xt, cb):
    nc = tc.nc

    consts = ctx.enter_context(tc.tile_pool(name="consts", bufs=1))
    cb_s = consts.tile([128, 428], BF, tag="cb")
    nc.sync.dma_start(cb_s[:], cb[:])
    id_s = cb_s[:, 0:128]
    ethh_s = cb_s[0:101, 128:228]
    ethl_s = cb_s[0:101, 228:328]
    e2_s = cb_s[0:K, 328:428]

    xp = ctx.enter_context(tc.tile_pool(name="xt", bufs=3))
    sp = ctx.enter_context(tc.tile_pool(name="scores", bufs=2, space="PSUM"))
    mvp = ctx.enter_context(tc.tile_pool(name="maxv", bufs=4))
    mp = ctx.enter_context(tc.tile_pool(name="mask", bufs=4))
    mtp = ctx.enter_context(tc.tile_pool(name="maskT", bufs=2, space="PSUM"))
    msp = ctx.enter_context(tc.tile_pool(name="maskTs", bufs=4))
    opp = ctx.enter_context(tc.tile_pool(name="outp", bufs=1, space="PSUM"))
    ogp = ctx.enter_context(tc.tile_pool(name="outg", bufs=3))

    xt_v = xt.rearrange("(g u) d n -> g d u n", u=GIN)  # [31, 101, 2, 2048]
    out_v = out.rearrange("(g u p w) d -> g p u w d", u=GOUT, p=128, w=NCH)

    xtiles = {}
    scores_t = {}
    mask_t = {}
    maskTs_t = {}
    outp_t = {}
    outg_t = {}

    def load(g):
        tl = xp.tile([101, GIN, 2 * ST], BF, tag="xt")
        if g == 0:
            for u in range(GIN):
                nc.sync.dma_start(out=tl[:, u], in_=xt_v[g, :, u])
        else:
            nc.sync.dma_start(out=tl[:], in_=xt_v[g])
        xtiles[g] = tl

    def S(t):
        g, u = divmod(t, GIN)
        tl = xtiles[g]
        sc = sp.tile([128, NCH, 128], F32, tag="scores")
        for c in range(NCH):
            hi = tl[:, u, bass.ts(c, 128)]
            lo = tl[:, u, ST + c * 128 : ST + (c + 1) * 128]
            nc.tensor.matmul(sc[:, c, 0:K], hi, ethh_s, start=True, stop=False)
            nc.tensor.matmul(sc[:, c, 0:K], hi, ethl_s, start=False, stop=False)
            nc.tensor.matmul(sc[:, c, 0:K], lo, ethh_s, start=False, stop=True)
        scores_t[t] = sc
        if u == GIN - 1:
            del xtiles[g]

    def RM(t):
        sc = scores_t.pop(t)
        mv = mvp.tile([128, NCH], F32, tag="maxv")
        nc.vector.tensor_reduce(
            mv[:], sc[:, :, 0:K], axis=mybir.AxisListType.X, op=mybir.AluOpType.max
        )
        mk = mp.tile([128, NCH, K], BF, tag="mask")
        mvv = mv[:].rearrange("p (f o) -> p f o", o=1)
        s_ap, m_ap = bass.broadcast_tensor_aps(sc[:, :, 0:K], mvv)
        nc.vector.tensor_tensor(out=mk[:], in0=s_ap, in1=m_ap, op=mybir.AluOpType.is_ge)
        mask_t[t] = mk

    def TC(t):
        mk = mask_t.pop(t)
        mt = mtp.tile([K, NCH, 128], BF, tag="maskT")
        for c in range(NCH):
            nc.tensor.transpose(mt[:, c], mk[:, c], id_s)
        ms = msp.tile([K, NCH, 128], BF, tag="maskTs")
        nc.scalar.copy(ms[:], mt[:])
        maskTs_t[t] = ms

    def G(t):
        ms = maskTs_t.pop(t)
        op_ = opp.tile([128, NCH, 128], F32, tag="outp")
        for c in range(NCH):
            nc.tensor.matmul(op_[:, c, 0:D], ms[:, c], e2_s, start=True, stop=True)
        outp_t[t] = op_

    def C2(t):
        g, u = divmod(t, GOUT)
        if u == 0:
            outg_t[g] = ogp.tile([128, GOUT, NCH, D], BF, tag="outg", name="outg")
        og = outg_t[g]
        op_ = outp_t.pop(t)
        nc.scalar.copy(og[:, u], op_[:, :, 0:D])
        if u == GOUT - 1:
            nc.sync.dma_start(out=out_v[g], in_=og[:])
            del outg_t[g]

    load(0)
    load(1)
    for i in range(T + 4):
        if i % GIN == 0:
            g = i // GIN + 2
            if g < NG:
                load(g)
        if 0 <= i - 2 < T:
            TC(i - 2)
        if 0 <= i - 3 < T:
            G(i - 3)
        if 0 <= i - 4 < T:
            C2(i - 4)
        if i < T:
            S(i)
        if 0 <= i - 1 < T:
            RM(i - 1)


def build_nc():
    nc = bacc.Bacc(
        "TRN2",
        target_bir_lowering=False,
        debug=False,
        enable_asserts=False,
        num_devices=N_CORES,
    )
    out = nc.dram_tensor("out", [NP, D], BF, kind="ExternalOutput").ap()
    xt = nc.dram_tensor("xt", [T, 101, 2 * ST], BF, kind="ExternalInput").ap()
    cb = nc.dram_tensor("cb", [128, 428], BF, kind="ExternalInput").ap()
    with tile.TileContext(nc) as tc:
        _vq_tile_kernel(tc, out, xt, cb)
    nc.compile()
    return nc


def prep_inputs(inputs: np.ndarray, embeddings: np.ndarray):
    """Host-side shard + layout prep. Returns in_maps for the 8 cores."""
    x = np.ascontiguousarray(inputs, dtype=np.float32)
    e = np.ascontiguousarray(embeddings, dtype=np.float32)

    ethf = np.zeros((101, K), dtype=np.float32)
    ethf[0:D] = e.T
    ethf[D] = (-0.5 * np.sum(e.astype(np.float64) ** 2, axis=1)).astype(np.float32)
    ethh = ethf.astype(bf16)
    ethl = (ethf - ethh.astype(np.float32)).astype(bf16)
    cb = np.zeros((128, 428), dtype=bf16)
    cb[:, 0:128] = np.eye(128, dtype=bf16)
    cb[0:101, 128:228] = ethh
    cb[0:101, 228:328] = ethl
    cb[0:K, 328:428] = e.astype(bf16)

    in_maps = []
    for i in range(N_CORES):
        xs = x[i * N_SHARD : (i + 1) * N_SHARD]
        xpad = np.zeros((NP, D), dtype=np.float32)
        xpad[:N_SHARD] = xs
        # device col c*128+p holds original row NCH*p+c of the super-tile
        v = xpad.reshape(T, 128, NCH, D)
        xtf = np.empty((T, 101, ST), dtype=np.float32)
        xtf[:, 0:D, :] = v.transpose(0, 3, 2, 1).reshape(T, D, ST)
        xtf[:, D, :] = 1.0
        xt = np.empty((T, 101, 2 * ST), dtype=bf16)
        xt[:, :, 0:ST] = xtf.astype(bf16)
        xt[:, :, ST:] = (xtf - xt[:, :, 0:ST].astype(np.float32)).astype(bf16)
        in_maps.append({"xt": xt, "cb": cb})
    return in_maps


_NC_CACHE = None


def kernel(inputs: np.ndarray, embeddings: np.ndarray) -> np.ndarray:
    global _NC_CACHE
    if _NC_CACHE is None:
        _NC_CACHE = build_nc()
    nc = _NC_CACHE
    in_maps = prep_inputs(inputs, embeddings)
    res = run_bass_kernel_spmd(nc, in_maps, core_ids=list(range(N_CORES)))
    shards = [res.results[i]["out"][:N_SHARD] for i in range(N_CORES)]
    full = np.concatenate(shards, axis=0)
    return np.ascontiguousarray(full.astype(np.float32))


# revision 15
# speedup vs baseline: 1.2219x; 1.0028x over previous
"""VQ codebook lookup (nn_VQ) on 8 TRN2 NeuronCores.

reference: idx = argmin_k ||x_n - e_k||^2 ; out = embeddings[idx]
Equivalent: idx = argmax_k (x_n . e_k - 0.5||e_k||^2)  (||x||^2 is constant per row)

Strategy v4 (data-parallel over N, codebook replicated):
  - Host: per core, shard x into [62500, 100] f32, pad to [63488, 100]
    (62 super-tiles of 1024 rows), pre-transpose each super-tile to
    [101, 1024] (row 100 = 1.0 bias-aug) with the 1024 columns stored in
    interleaved order (device col c*128+p holds original row 8p+c) so the
    output DMA sees contiguous multi-row elements, then split into bf16
    hi/lo halves: xt [62, 2, 101, 1024] bf16. Codebook constants:
    eth_h/eth_l [101, 100] bf16 hi/lo of [e.T; -0.5||e||^2],
    e2 [100, 100] bf16, identity [128, 128] bf16.
  - Device, per 1024-row super-tile:
      scores: 8 chunks x 3 accumulating bf16 matmuls (x_hi.eth_h + x_hi.eth_l
        + x_lo.eth_h) -> PSUM [128, 8, 128] f32 (chunks strided 512B to stay
        bank-aligned, 100 real cols each)
      DVE reduce_max + is_ge (broadcast AP) -> exact one-hot bf16 mask
      8x PE-transpose (bf16, identity moving) -> maskT PSUM [100, 8, 128]
      Act copy -> maskTs SBUF
      gather: 8x bf16 matmul (maskT chunk stationary, e2 moving)
        -> out rows PSUM [128, 8, 128] f32 (512B-strided chunks)
      Act copy (casts f32->bf16) into a 2-tile output group buffer;
      one DMA out per 2 tiles (contiguous 1600B elems via the interleave).
  - DMAs: input loads grouped 2 super-tiles per instruction (plain loads of
    the host-pretransposed layout; elem = 2KB). Output bf16, upcast on host.
  - Software pipeline: per iteration i emit S(i), RM(i-1), TC(i-2), C2(i-4),
    G(i-3) so the PE never stalls on the DVE argmax or the copies.
"""

import sys

sys.path.insert(0, "/opt/trn_rl_repo")
from contextlib import ExitStack

import ml_dtypes
import numpy as np

import concourse.bass as bass
import concourse.bacc as bacc
import concourse.tile as tile
from concourse import mybir
from concourse._compat import with_exitstack
from concourse.bass_utils import run_bass_kernel_spmd

F32 = mybir.dt.float32
BF = mybir.dt.bfloat16
bf16 = ml_dtypes.bfloat16

N_TOTAL = 500_000
D = 100
K = 100
N_CORES = 8
ST = 1024  # rows per super-tile
NCH = ST // 128  # 8 chunks
N_SHARD = N_TOTAL // N_CORES  # 62500
T = -(-N_SHARD // ST)  # 62 super-tiles
NP = T * ST  # 63488 padded rows per core
GIN = 2  # super-tiles per input DMA
GOUT = 2  # super-tiles per output DMA
NG = T // GIN  # 31 groups


@with_exitstack
def _vq_tile_kernel(ctx: ExitStack, tc: tile.TileContext, out, xt, cb):
    nc = tc.nc

    consts = ctx.enter_context(tc.tile_pool(name="consts", bufs=1))
    cb_s = consts.tile([128, 428], BF, tag="cb")
    nc.sync.dma_start(cb_s[:], cb[:])
    id_s = cb_s[:, 0:128]
    ethh_s = cb_s[0:101, 128:228]
    ethl_s = cb_s[0:101, 228:328]
    e2_s = cb_s[0:K, 328:428]

    xp = ctx.enter_context(tc.tile_pool(name="xt", bufs=4))
    sp = ctx.enter_context(tc.tile_pool(name="scores", bufs=2, space="PSUM"))
    mvp = ctx.enter_context(tc.tile_pool(name="maxv", bufs=4))
    mp = ctx.enter_context(tc.tile_pool(name="mask", bufs=4))
    mtp = ctx.enter_context(tc.tile_pool(name="maskT", bufs=2, space="PSUM"))
    msp = ctx.enter_context(tc.tile_pool(name="maskTs", bufs=4))
    opp = ctx.enter_context(tc.tile_pool(name="outp", bufs=1, space="PSUM"))
    ogp = ctx.enter_context(tc.tile_pool(name="outg", bufs=3))

    xt_v = xt.rearrange("(g u) d n -> g d u n", u=GIN)  # [31, 101, 2, 2048]
    out_v = out.rearrange("(g u p w) d -> g p u w d", u=GOUT, p=128, w=NCH)

    xtiles = {}
    scores_t = {}
    mask_t = {}
    maskTs_t = {}
    outp_t = {}
    outg_t = {}

    def load(g):
        tl = xp.tile([101, GIN, 2 * ST], BF, tag="xt")
        if g <= 1:
            for u in range(GIN):
                nc.sync.dma_start(out=tl[:, u], in_=xt_v[g, :, u])
        else:
            nc.sync.dma_start(out=tl[:], in_=xt_v[g])
        xtiles[g] = tl

    def S(t):
        g, u = divmod(t, GIN)
        tl = xtiles[g]
        sc = sp.tile([128, NCH, 128], F32, tag="scores")
        for c in range(NCH):
            hi = tl[:, u, bass.ts(c, 128)]
            lo = tl[:, u, ST + c * 128 : ST + (c + 1) * 128]
            nc.tensor.matmul(sc[:, c, 0:K], hi, ethh_s, start=True, stop=False)
            nc.tensor.matmul(sc[:, c, 0:K], hi, ethl_s, start=False, stop=False)
            nc.tensor.matmul(sc[:, c, 0:K], lo, ethh_s, start=False, stop=True)
        scores_t[t] = sc
        if u == GIN - 1:
            del xtiles[g]

    def RM(t):
        sc = scores_t.pop(t)
        mv = mvp.tile([128, NCH], F32, tag="maxv")
        nc.vector.tensor_reduce(
            mv[:], sc[:, :, 0:K], axis=mybir.AxisListType.X, op=mybir.AluOpType.max
        )
        mk = mp.tile([128, NCH, K], BF, tag="mask")
        mvv = mv[:].rearrange("p (f o) -> p f o", o=1)
        s_ap, m_ap = bass.broadcast_tensor_aps(sc[:, :, 0:K], mvv)
        nc.vector.tensor_tensor(out=mk[:], in0=s_ap, in1=m_ap, op=mybir.AluOpType.is_ge)
        mask_t[t] = mk

    def TC(t):
        mk = mask_t.pop(t)
        mt = mtp.tile([K, NCH, 128], BF, tag="maskT")
        for c in range(NCH):
            nc.tensor.transpose(mt[:, c], mk[:, c], id_s)
        ms = msp.tile([K, NCH, 128], BF, tag="maskTs")
        nc.scalar.copy(ms[:], mt[:])
        maskTs_t[t] = ms

    def G(t):
        ms = maskTs_t.pop(t)
        op_ = opp.tile([128, NCH, 128], F32, tag="outp")
        for c in range(NCH):
            nc.tensor.matmul(op_[:, c, 0:D], ms[:, c], e2_s, start=True, stop=True)
        outp_t[t] = op_

    def C2(t):
        g, u = divmod(t, GOUT)
        if u == 0:
            outg_t[g] = ogp.tile([128, GOUT, NCH, D], BF, tag="outg", name="outg")
        og = outg_t[g]
        op_ = outp_t.pop(t)
        nc.scalar.copy(og[:, u], op_[:, :, 0:D])
        if g == NG - 1:
            nc.sync.dma_start(out=out_v[g, :, u], in_=og[:, u])
            if u == GOUT - 1:
                del outg_t[g]
        elif u == GOUT - 1:
            nc.sync.dma_start(out=out_v[g], in_=og[:])
            del outg_t[g]

    load(0)
    load(1)
    for i in range(T + 4):
        if i % GIN == 0:
            g = i // GIN + 2
            if g < NG:
                load(g)
        if 0 <= i - 2 < T:
            TC(i - 2)
        if 0 <= i - 3 < T:
            G(i - 3)
        if 0 <= i - 4 < T:
            C2(i - 4)
        if i < T:
            S(i)
        if 0 <= i - 1 < T:
            RM(i - 1)


def build_nc():
    nc = bacc.Bacc(
        "TRN2",
        target_bir_lowering=False,
        debug=False,
        enable_asserts=False,
        num_devices=N_CORES,
    )
    out = nc.dram_tensor("out", [NP, D], BF, kind="ExternalOutput").ap()
    xt = nc.dram_tensor("xt", [T, 101, 2 * ST], BF, kind="ExternalInput").ap()
    cb = nc.dram_tensor("cb", [128, 428], BF, kind="ExternalInput").ap()
    with tile.TileContext(nc) as tc:
        _vq_tile_kernel(tc, out, xt, cb)
    nc.compile()
    return nc


def prep_inputs(inputs: np.ndarray, embeddings: np.ndarray):
    """Host-side shard + layout prep. Returns in_maps for the 8 cores."""
    x = np.ascontiguousarray(inputs, dtype=np.float32)
    e = np.ascontiguousarray(embeddings, dtype=np.float32)

    ethf = np.zeros((101, K), dtype=np.float32)
    ethf[0:D] = e.T
    ethf[D] = (-0.5 * np.sum(e.astype(np.float64) ** 2, axis=1)).astype(np.float32)
    ethh = ethf.astype(bf16)
    ethl = (ethf - ethh.astype(np.float32)).astype(bf16)
    cb = np.zeros((128, 428), dtype=bf16)
    cb[:, 0:128] = np.eye(128, dtype=bf16)
    cb[0:101, 128:228] = ethh
    cb[0:101, 228:328] = ethl
    cb[0:K, 328:428] = e.astype(bf16)

    in_maps = []
    for i in range(N_CORES):
        xs = x[i * N_SHARD : (i + 1) * N_SHARD]
        xpad = np.zeros((NP, D), dtype=np.float32)
        xpad[:N_SHARD] = xs
        # device col c*128+p holds original row NCH*p+c of the super-tile
        v = xpad.reshape(T, 128, NCH, D)
        xtf = np.empty((T, 101, ST), dtype=np.float32)
        xtf[:, 0:D, :] = v.transpose(0, 3, 2, 1).reshape(T, D, ST)
        xtf[:, D, :] = 1.0
        xt = np.empty((T, 101, 2 * ST), dtype=bf16)
        xt[:, :, 0:ST] = xtf.astype(bf16)
        xt[:, :, ST:] = (xtf - xt[:, :, 0:ST].astype(np.float32)).astype(bf16)
        in_maps.append({"xt": xt, "cb": cb})
    return in_maps


_NC_CACHE = None


def kernel(inputs: np.ndarray, embeddings: np.ndarray) -> np.ndarray:
    global _NC_CACHE
    if _NC_CACHE is None:
        _NC_CACHE = build_nc()
    nc = _NC_CACHE
    in_maps = prep_inputs(inputs, embeddings)
    res = run_bass_kernel_spmd(nc, in_maps, core_ids=list(range(N_CORES)))
    shards = [res.results[i]["out"][:N_SHARD] for i in range(N_CORES)]
    full = np.concatenate(shards, axis=0)
    return np.ascontiguousarray(full.astype(np.float32))


# revision 17
# speedup vs baseline: 1.2232x; 1.0010x over previous
"""VQ codebook lookup (nn_VQ) on 8 TRN2 NeuronCores.

reference: idx = argmin_k ||x_n - e_k||^2 ; out = embeddings[idx]
Equivalent: idx = argmax_k (x_n . e_k - 0.5||e_k||^2)  (||x||^2 is constant per row)

Strategy v4 (data-parallel over N, codebook replicated):
  - Host: per core, shard x into [62500, 100] f32, pad to [63488, 100]
    (62 super-tiles of 1024 rows), pre-transpose each super-tile to
    [101, 1024] (row 100 = 1.0 bias-aug) with the 1024 columns stored in
    interleaved order (device col c*128+p holds original row 8p+c) so the
    output DMA sees contiguous multi-row elements, then split into bf16
    hi/lo halves: xt [62, 2, 101, 1024] bf16. Codebook constants:
    eth_h/eth_l [101, 100] bf16 hi/lo of [e.T; -0.5||e||^2],
    e2 [100, 100] bf16, identity [128, 128] bf16.
  - Device, per 1024-row super-tile:
      scores: 8 chunks x 3 accumulating bf16 matmuls (x_hi.eth_h + x_hi.eth_l
        + x_lo.eth_h) -> PSUM [128, 8, 128] f32 (chunks strided 512B to stay
        bank-aligned, 100 real cols each)
      DVE reduce_max + is_ge (broadcast AP) -> exact one-hot bf16 mask
      8x PE-transpose (bf16, identity moving) -> maskT PSUM [100, 8, 128]
      Act copy -> maskTs SBUF
      gather: 8x bf16 matmul (maskT chunk stationary, e2 moving)
        -> out rows PSUM [128, 8, 128] f32 (512B-strided chunks)
      Act copy (casts f32->bf16) into a 2-tile output group buffer;
      one DMA out per 2 tiles (contiguous 1600B elems via the interleave).
  - DMAs: input loads grouped 2 super-tiles per instruction (plain loads of
    the host-pretransposed layout; elem = 2KB). Output bf16, upcast on host.
  - Software pipeline: per iteration i emit S(i), RM(i-1), TC(i-2), C2(i-4),
    G(i-3) so the PE never stalls on the DVE argmax or the copies.
"""

import sys

sys.path.insert(0, "/opt/trn_rl_repo")
from contextlib import ExitStack

import ml_dtypes
import numpy as np

import concourse.bass as bass
import concourse.bacc as bacc
import concourse.tile as tile
from concourse import mybir
from concourse._compat import with_exitstack
from concourse.bass_utils import run_bass_kernel_spmd

F32 = mybir.dt.float32
BF = mybir.dt.bfloat16
bf16 = ml_dtypes.bfloat16

N_TOTAL = 500_000
D = 100
K = 100
N_CORES = 8
ST = 1024  # rows per super-tile
NCH = ST // 128  # 8 chunks
N_SHARD = N_TOTAL // N_CORES  # 62500
T = -(-N_SHARD // ST)  # 62 super-tiles
NP = T * ST  # 63488 padded rows per core
GIN = 2  # super-tiles per input DMA
GOUT = 2  # super-tiles per output DMA
NG = T // GIN  # 31 groups


@with_exitstack
def _vq_tile_kernel(ctx: ExitStack, tc: tile.TileContext, out, xt, cb):
    nc = tc.nc

    consts = ctx.enter_context(tc.tile_pool(name="consts", bufs=1))
    cb_s = consts.tile([128, 428], BF, tag="cb")
    nc.sync.dma_start(cb_s[:], cb[:])
    id_s = cb_s[:, 0:128]
    ethh_s = cb_s[0:101, 128:228]
    ethl_s = cb_s[0:101, 228:328]
    e2_s = cb_s[0:K, 328:428]

    xp = ctx.enter_context(tc.tile_pool(name="xt", bufs=4))
    sp = ctx.enter_context(tc.tile_pool(name="scores", bufs=2, space="PSUM"))
    mvp = ctx.enter_context(tc.tile_pool(name="maxv", bufs=4))
    mp = ctx.enter_context(tc.tile_pool(name="mask", bufs=4))
    mtp = ctx.enter_context(tc.tile_pool(name="maskT", bufs=2, space="PSUM"))
    msp = ctx.enter_context(tc.tile_pool(name="maskTs", bufs=4))
    opp = ctx.enter_context(tc.tile_pool(name="outp", bufs=1, space="PSUM"))
    ogp = ctx.enter_context(tc.tile_pool(name="outg", bufs=3))

    xt_v = xt.rearrange("(g u) d n -> g d u n", u=GIN)  # [31, 101, 2, 2048]
    out_v = out.rearrange("(g u p w) d -> g p u w d", u=GOUT, p=128, w=NCH)

    xtiles = {}
    scores_t = {}
    mask_t = {}
    maskTs_t = {}
    outp_t = {}
    outg_t = {}

    def load(g):
        tl = xp.tile([101, GIN, 2 * ST], BF, tag="xt")
        if g <= 1:
            for u in range(GIN):
                nc.sync.dma_start(out=tl[:, u], in_=xt_v[g, :, u])
        else:
            nc.sync.dma_start(out=tl[:], in_=xt_v[g])
        xtiles[g] = tl

    def S(t):
        g, u = divmod(t, GIN)
        tl = xtiles[g]
        sc = sp.tile([128, NCH, 128], F32, tag="scores")
        for c in range(NCH):
            hi = tl[:, u, bass.ts(c, 128)]
            lo = tl[:, u, ST + c * 128 : ST + (c + 1) * 128]
            nc.tensor.matmul(sc[:, c, 0:K], hi, ethh_s, start=True, stop=False)
            nc.tensor.matmul(sc[:, c, 0:K], hi, ethl_s, start=False, stop=False)
            nc.tensor.matmul(sc[:, c, 0:K], lo, ethh_s, start=False, stop=True)
        scores_t[t] = sc
        if u == GIN - 1:
            del xtiles[g]

    def RM(t):
        sc = scores_t.pop(t)
        mv = mvp.tile([128, NCH], F32, tag="maxv")
        nc.vector.tensor_reduce(
            mv[:], sc[:, :, 0:K], axis=mybir.AxisListType.X, op=mybir.AluOpType.max
        )
        mk = mp.tile([128, NCH, K], BF, tag="mask")
        mvv = mv[:].rearrange("p (f o) -> p f o", o=1)
        s_ap, m_ap = bass.broadcast_tensor_aps(sc[:, :, 0:K], mvv)
        nc.vector.tensor_tensor(out=mk[:], in0=s_ap, in1=m_ap, op=mybir.AluOpType.is_ge)
        mask_t[t] = mk

    def TC(t):
        mk = mask_t.pop(t)
        mt = mtp.tile([K, NCH, 128], BF, tag="maskT")
        for c in range(NCH):
            nc.tensor.transpose(mt[:, c], mk[:, c], id_s)
        ms = msp.tile([K, NCH, 128], BF, tag="maskTs")
        nc.scalar.copy(ms[:], mt[:])
        maskTs_t[t] = ms

    def G(t):
        ms = maskTs_t.pop(t)
        op_ = opp.tile([128, NCH, 128], F32, tag="outp")
        for c in range(NCH):
            nc.tensor.matmul(op_[:, c, 0:D], ms[:, c], e2_s, start=True, stop=True)
        outp_t[t] = op_

    def C2(t):
        g, u = divmod(t, GOUT)
        if u == 0:
            outg_t[g] = ogp.tile([128, GOUT, NCH, D], BF, tag="outg", name="outg")
        og = outg_t[g]
        op_ = outp_t.pop(t)
        nc.scalar.copy(og[:, u], op_[:, :, 0:D])
        nc.sync.dma_start(out=out_v[g, :, u], in_=og[:, u])
        if u == GOUT - 1:
            del outg_t[g]

    load(0)
    load(1)
    for i in range(T + 4):
        if i % GIN == 0:
            g = i // GIN + 2
            if g < NG:
                load(g)
        if 0 <= i - 2 < T:
            TC(i - 2)
        if 0 <= i - 3 < T:
            G(i - 3)
        if 0 <= i - 4 < T:
            C2(i - 4)
        if i < T:
            S(i)
        if 0 <= i - 1 < T:
            RM(i - 1)


def build_nc():
    nc = bacc.Bacc(
        "TRN2",
        target_bir_lowering=False,
        debug=False,
        enable_asserts=False,
        num_devices=N_CORES,
    )
    out = nc.dram_tensor("out", [NP, D], BF, kind="ExternalOutput").ap()
    xt = nc.dram_tensor("xt", [T, 101, 2 * ST], BF, kind="ExternalInput").ap()
    cb = nc.dram_tensor("cb", [128, 428], BF, kind="ExternalInput").ap()
    with tile.TileContext(nc) as tc:
        _vq_tile_kernel(tc, out, xt, cb)
    nc.compile()
    return nc


def prep_inputs(inputs: np.ndarray, embeddings: np.ndarray):
    """Host-side shard + layout prep. Returns in_maps for the 8 cores."""
    x = np.ascontiguousarray(inputs, dtype=np.float32)
    e = np.ascontiguousarray(embeddings, dtype=np.float32)

    ethf = np.zeros((101, K), dtype=np.float32)
    ethf[0:D] = e.T
    ethf[D] = (-0.5 * np.sum(e.astype(np.float64) ** 2, axis=1)).astype(np.float32)
    ethh = ethf.astype(bf16)
    ethl = (ethf - ethh.astype(np.float32)).astype(bf16)
    cb = np.zeros((128, 428), dtype=bf16)
    cb[:, 0:128] = np.eye(128, dtype=bf16)
    cb[0:101, 128:228] = ethh
    cb[0:101, 228:328] = ethl
    cb[0:K, 328:428] = e.astype(bf16)

    in_maps = []
    for i in range(N_CORES):
        xs = x[i * N_SHARD : (i + 1) * N_SHARD]
        xpad = np.zeros((NP, D), dtype=np.float32)
        xpad[:N_SHARD] = xs
        # device col c*128+p holds original row NCH*p+c of the super-tile
        v = xpad.reshape(T, 128, NCH, D)
        xtf = np.empty((T, 101, ST), dtype=np.float32)
        xtf[:, 0:D, :] = v.transpose(0, 3, 2, 1).reshape(T, D, ST)
        xtf[:, D, :] = 1.0
        xt = np.empty((T, 101, 2 * ST), dtype=bf16)
        xt[:, :, 0:ST] = xtf.astype(bf16)
        xt[:, :, ST:] = (xtf - xt[:, :, 0:ST].astype(np.float32)).astype(bf16)
        in_maps.append({"xt": xt, "cb": cb})
    return in_maps


_NC_CACHE = None


def kernel(inputs: np.ndarray, embeddings: np.ndarray) -> np.ndarray:
    global _NC_CACHE
    if _NC_CACHE is None:
        _NC_CACHE = build_nc()
    nc = _NC_CACHE
    in_maps = prep_inputs(inputs, embeddings)
    res = run_bass_kernel_spmd(nc, in_maps, core_ids=list(range(N_CORES)))
    shards = [res.results[i]["out"][:N_SHARD] for i in range(N_CORES)]
    full = np.concatenate(shards, axis=0)
    return np.ascontiguousarray(full.astype(np.float32))


# revision 24
# speedup vs baseline: 1.2454x; 1.0182x over previous
"""VQ codebook lookup (nn_VQ) on 8 TRN2 NeuronCores.

reference: idx = argmin_k ||x_n - e_k||^2 ; out = embeddings[idx]
Equivalent: idx = argmax_k (x_n . e_k - 0.5||e_k||^2)  (||x||^2 is constant per row)

Strategy v4 (data-parallel over N, codebook replicated):
  - Host: per core, shard x into [62500, 100] f32, pad to [63488, 100]
    (62 super-tiles of 1024 rows), pre-transpose each super-tile to
    [101, 1024] (row 100 = 1.0 bias-aug) with the 1024 columns stored in
    interleaved order (device col c*128+p holds original row 8p+c) so the
    output DMA sees contiguous multi-row elements, then split into bf16
    hi/lo halves: xt [62, 2, 101, 1024] bf16. Codebook constants:
    eth_h/eth_l [101, 100] bf16 hi/lo of [e.T; -0.5||e||^2],
    e2 [100, 100] bf16, identity [128, 128] bf16.
  - Device, per 1024-row super-tile:
      scores: 8 chunks x 3 accumulating bf16 matmuls (x_hi.eth_h + x_hi.eth_l
        + x_lo.eth_h) -> PSUM [128, 8, 128] f32 (chunks strided 512B to stay
        bank-aligned, 100 real cols each)
      DVE reduce_max + is_ge (broadcast AP) -> exact one-hot bf16 mask
      8x PE-transpose (bf16, identity moving) -> maskT PSUM [100, 8, 128]
      Act copy -> maskTs SBUF
      gather: 8x bf16 matmul (maskT chunk stationary, e2 moving)
        -> out rows PSUM [128, 8, 128] f32 (512B-strided chunks)
      Act copy (casts f32->bf16) into a 2-tile output group buffer;
      one DMA out per 2 tiles (contiguous 1600B elems via the interleave).
  - DMAs: input loads grouped 2 super-tiles per instruction (plain loads of
    the host-pretransposed layout; elem = 2KB). Output bf16, upcast on host.
  - Software pipeline: per iteration i emit S(i), RM(i-1), TC(i-2), C2(i-4),
    G(i-3) so the PE never stalls on the DVE argmax or the copies.
"""

import sys

sys.path.insert(0, "/opt/trn_rl_repo")
from contextlib import ExitStack

import ml_dtypes
import numpy as np

import concourse.bass as bass
import concourse.bacc as bacc
import concourse.tile as tile
from concourse import mybir
from concourse._compat import with_exitstack
from concourse.bass_utils import run_bass_kernel_spmd

F32 = mybir.dt.float32
BF = mybir.dt.bfloat16
bf16 = ml_dtypes.bfloat16

N_TOTAL = 500_000
D = 100
K = 100
N_CORES = 8
ST = 1024  # rows per super-tile
NCH = ST // 128  # 8 chunks
N_SHARD = N_TOTAL // N_CORES  # 62500
T = -(-N_SHARD // ST)  # 62 super-tiles
NP = T * ST  # 63488 padded rows per core
GIN = 2  # super-tiles per input DMA
GOUT = 2  # super-tiles per output DMA
NG = T // GIN  # 31 groups


@with_exitstack
def _vq_tile_kernel(ctx: ExitStack, tc: tile.TileContext, out, xt, cb):
    nc = tc.nc

    consts = ctx.enter_context(tc.tile_pool(name="consts", bufs=1))
    cb_s = consts.tile([128, 428], BF, tag="cb")
    nc.sync.dma_start(cb_s[:], cb[:])
    id_s = cb_s[:, 0:128]
    ethh_s = cb_s[0:101, 128:228]
    ethl_s = cb_s[0:101, 228:328]
    e2_s = cb_s[0:K, 328:428]

    xp = ctx.enter_context(tc.tile_pool(name="xt", bufs=4))
    sp = ctx.enter_context(tc.tile_pool(name="scores", bufs=2, space="PSUM"))
    mvp = ctx.enter_context(tc.tile_pool(name="maxv", bufs=4))
    mp = ctx.enter_context(tc.tile_pool(name="mask", bufs=4))
    mtp = ctx.enter_context(tc.tile_pool(name="maskT", bufs=2, space="PSUM"))
    msp = ctx.enter_context(tc.tile_pool(name="maskTs", bufs=4))
    opp = ctx.enter_context(tc.tile_pool(name="outp", bufs=1, space="PSUM"))
    ogp = ctx.enter_context(tc.tile_pool(name="outg", bufs=6))

    xt_v = xt.rearrange("(g u) d n -> g d u n", u=GIN)  # [31, 101, 2, 2048]
    out_v = out.rearrange("(g u p w) d -> g p u w d", u=GOUT, p=128, w=NCH)

    xtiles = {}
    scores_t = {}
    mask_t = {}
    maskTs_t = {}
    outp_t = {}
    outg_t = {}

    def load(g):
        tl = xp.tile([101, GIN, 2 * ST], BF, tag="xt")
        if g <= 1:
            for u in range(GIN):
                nc.sync.dma_start(out=tl[:, u], in_=xt_v[g, :, u])
        else:
            nc.sync.dma_start(out=tl[:], in_=xt_v[g])
        xtiles[g] = tl

    def S(t):
        g, u = divmod(t, GIN)
        tl = xtiles[g]
        sc = sp.tile([128, NCH, 128], F32, tag="scores")
        for c in range(NCH):
            hi = tl[:, u, bass.ts(c, 128)]
            lo = tl[:, u, ST + c * 128 : ST + (c + 1) * 128]
            nc.tensor.matmul(sc[:, c, 0:K], hi, ethh_s, start=True, stop=False)
            nc.tensor.matmul(sc[:, c, 0:K], hi, ethl_s, start=False, stop=False)
            nc.tensor.matmul(sc[:, c, 0:K], lo, ethh_s, start=False, stop=True)
        scores_t[t] = sc
        if u == GIN - 1:
            del xtiles[g]

    def RM(t):
        sc = scores_t.pop(t)
        mv = mvp.tile([128, NCH], F32, tag="maxv")
        nc.vector.tensor_reduce(
            mv[:], sc[:, :, 0:K], axis=mybir.AxisListType.X, op=mybir.AluOpType.max
        )
        mk = mp.tile([128, NCH, K], BF, tag="mask")
        mvv = mv[:].rearrange("p (f o) -> p f o", o=1)
        s_ap, m_ap = bass.broadcast_tensor_aps(sc[:, :, 0:K], mvv)
        nc.vector.tensor_tensor(out=mk[:], in0=s_ap, in1=m_ap, op=mybir.AluOpType.is_ge)
        mask_t[t] = mk

    def TC(t):
        mk = mask_t.pop(t)
        mt = mtp.tile([K, NCH, 128], BF, tag="maskT")
        for c in range(NCH):
            nc.tensor.transpose(mt[:, c], mk[:, c], id_s)
        ms = msp.tile([K, NCH, 128], BF, tag="maskTs")
        nc.scalar.copy(ms[:].bitcast(F32), mt[:].bitcast(F32))
        maskTs_t[t] = ms

    def G(t):
        ms = maskTs_t.pop(t)
        op_ = opp.tile([128, NCH, 128], F32, tag="outp")
        for c in range(NCH):
            nc.tensor.matmul(op_[:, c, 0:D], ms[:, c], e2_s, start=True, stop=True)
        outp_t[t] = op_

    def C2(t):
        g, u = divmod(t, GOUT)
        if u == 0:
            outg_t[g] = ogp.tile([128, GOUT, NCH, D], BF, tag="outg", name="outg")
        og = outg_t[g]
        op_ = outp_t.pop(t)
        nc.scalar.copy(og[:, u], op_[:, :, 0:D])
        nc.sync.dma_start(out=out_v[g, :, u], in_=og[:, u])
        if u == GOUT - 1:
            del outg_t[g]

    load(0)
    load(1)
    for i in range(T + 4):
        if i % GIN == 0:
            g = i // GIN + 2
            if g < NG:
                load(g)
        if 0 <= i - 2 < T:
            TC(i - 2)
        if 0 <= i - 3 < T:
            G(i - 3)
        if 0 <= i - 4 < T:
            C2(i - 4)
        if i < T:
            S(i)
        if 0 <= i - 1 < T:
            RM(i - 1)


def build_nc():
    nc = bacc.Bacc(
        "TRN2",
        target_bir_lowering=False,
        debug=False,
        enable_asserts=False,
        num_devices=N_CORES,
    )
    out = nc.dram_tensor("out", [NP, D], BF, kind="ExternalOutput").ap()
    xt = nc.dram_tensor("xt", [T, 101, 2 * ST], BF, kind="ExternalInput").ap()
    cb = nc.dram_tensor("cb", [128, 428], BF, kind="ExternalInput").ap()
    with tile.TileContext(nc) as tc:
        _vq_tile_kernel(tc, out, xt, cb)
    nc.compile()
    return nc


def prep_inputs(inputs: np.ndarray, embeddings: np.ndarray):
    """Host-side shard + layout prep. Returns in_maps for the 8 cores."""
    x = np.ascontiguousarray(inputs, dtype=np.float32)
    e = np.ascontiguousarray(embeddings, dtype=np.float32)

    ethf = np.zeros((101, K), dtype=np.float32)
    ethf[0:D] = e.T
    ethf[D] = (-0.5 * np.sum(e.astype(np.float64) ** 2, axis=1)).astype(np.float32)
    ethh = ethf.astype(bf16)
    ethl = (ethf - ethh.astype(np.float32)).astype(bf16)
    cb = np.zeros((128, 428), dtype=bf16)
    cb[:, 0:128] = np.eye(128, dtype=bf16)
    cb[0:101, 128:228] = ethh
    cb[0:101, 228:328] = ethl
    cb[0:K, 328:428] = e.astype(bf16)

    in_maps = []
    for i in range(N_CORES):
        xs = x[i * N_SHARD : (i + 1) * N_SHARD]
        xpad = np.zeros((NP, D), dtype=np.float32)
        xpad[:N_SHARD] = xs
        # device col c*128+p holds original row NCH*p+c of the super-tile
        v = xpad.reshape(T, 128, NCH, D)
        xtf = np.empty((T, 101, ST), dtype=np.float32)
        xtf[:, 0:D, :] = v.transpose(0, 3, 2, 1).reshape(T, D, ST)
        xtf[:, D, :] = 1.0
        xt = np.empty((T, 101, 2 * ST), dtype=bf16)
        xt[:, :, 0:ST] = xtf.astype(bf16)
        xt[:, :, ST:] = (xtf - xt[:, :, 0:ST].astype(np.float32)).astype(bf16)
        in_maps.append({"xt": xt, "cb": cb})
    return in_maps


_NC_CACHE = None


def kernel(inputs: np.ndarray, embeddings: np.ndarray) -> np.ndarray:
    global _NC_CACHE
    if _NC_CACHE is None:
        _NC_CACHE = build_nc()
    nc = _NC_CACHE
    in_maps = prep_inputs(inputs, embeddings)
    res = run_bass_kernel_spmd(nc, in_maps, core_ids=list(range(N_CORES)))
    shards = [res.results[i]["out"][:N_SHARD] for i in range(N_CORES)]
    full = np.concatenate(shards, axis=0)
    return np.ascontiguousarray(full.astype(np.float32))
